# revision 38
# baseline (speedup 1.0000x reference)
"""Multi-head causal attention (B=4, T=2048, D=1024, H=16) on 8 TRN2 NeuronCores.

Sharding: data-parallel over batch (4) x tensor-parallel over heads (2 groups
of 8). Core c handles batch c//2, head-group c%2. Partial out-projections are
pairwise-summed on host.

Datapath (v2):
- Q/K/V projections run as 3-term fp8 residual matmuls in DoubleRow mode
  (x and W shipped from host as e4m3 hi+lo splits, W prescaled by 32):
  (Wh+Wl).T xh + Wh.T xl per kd-pair, 0.75x the bf16 row count at ~bf16
  accuracy. 12 DoubleRow matmuls per [128-out, 256-token] tile.
- K is stored as single-level fp8 (the one budgeted quantization, damped by
  small score magnitudes); Q is stored as an fp8 hi+lo pair. QK^T then runs
  in DoubleRow: lhsT = K dup'd via a stride-0 ktile dim, rhs = (q_hi, q_lo),
  0.5F cycles per (key-block, head) - half the bf16 cost, Q effectively exact.
- exp on Act (bf16 probs), triangular-mask multiplies on Pool.
- AV runs *swapped*: probs block [128 keys, 128 queries] stationary, V
  [128 keys, 64] + ones column [128, 1] moving, psum accumulates
  [query, feat] over key blocks: 65 moving rows per (key block, query block,
  head) vs 128 in the probs-moving orientation. Denominators land as
  per-query-partition scalars: one reciprocal_approx over [128, 8] and one
  stride-0-broadcast tensor_tensor multiply drain per unit replace the whole
  row-copy/reciprocal/partition-broadcast pipeline of the bf16 baseline.
- The drained attention output [query, feat] is DMA-transposed (128x128
  tiles) into [feat, token] for a bf16 out-projection (baseline o_step);
  each chunk's out-projections are scheduled >=1 unit after the last
  contributing transpose (the PE is in-order: a filler emitted before its
  inputs are ready stalls everything behind it).
- In rounds 2-3 (where exp is the per-unit bottleneck) every 3rd
  off-diagonal key block's exp runs as a Schraudolph fast-exp off Act:
  y = s*(scale*log2e*2^23) + (127-sigma)*2^23 on DVE, f32->i32 value
  convert on Pool, and the AV matmul consumes the probs as the strided
  bf16 (high-half) view of those i32 words - no separate convert pass.

Engine busy (cost model): PE ~147us, Act ~139us, DVE ~92us, Pool ~45us;
measured 204832 ns vs the 254172 ns bf16 baseline. Rel err 1.46e-2
on hardware (gate 2e-2).

Hazards learned the hard way (do not regress):
- fp8 DMAs with 256B contiguous runs pay a 2x latency multiplier; keep
  >=512B innermost runs (x is loaded per-512-token round, weights
  full-width).
- PSUM accumulation groups must not share a 2KB bank with concurrently
  accumulating groups unless starts/stops are ordered (av gets one bank
  with 256B-aligned groups + a single start; dn its own bank).
- Emission order around ensure_v/the unit tails is correctness-sensitive
  on hardware (a v_step pull-forward change produced deterministic NaN in
  chunk-0 rows); validate any reordering there with a full run.
"""

import sys

if "/opt/trn_rl_repo" not in sys.path:
    sys.path.insert(0, "/opt/trn_rl_repo")

import ml_dtypes
import numpy as np

import concourse.bass as bass
import concourse.mybir as mybir
from concourse import bacc
from concourse.bass import MemorySpace
from concourse.tile import TileContext

B, T, D = 4, 2048, 1024
H, DH = 16, 64
HG = 8          # heads per core
GW = HG * DH    # group width = 512
P = 128
KD = D // P     # 8 contraction chunks
NTB = T // P    # 16 key blocks of 128
N_CORES = 8
LAG = 9      # kb's between exp issue and AV consumption
PULL = 3     # filler steps pulled forward per kb
WS = 32.0    # host-side weight prescale for fp8 (subnormal avoidance)

F32 = mybir.dt.float32
BF16 = mybir.dt.bfloat16
FP8 = mybir.dt.float8e4
U16 = mybir.dt.uint16
DR = mybir.MatmulPerfMode.DoubleRow


def build_nc():
    nc = bacc.Bacc()

    xs_in = {}
    for nm in ("xq", "xk", "xv"):
        for lv in ("h", "l"):
            xs_in[nm + lv] = nc.dram_tensor(nm + lv, [D, T], FP8, kind="ExternalInput")
    ws_in = {}
    for nm in ("wq", "wk", "wv"):
        for lv in ("h", "l"):
            ws_in[nm + lv] = nc.dram_tensor(nm + lv, [D, GW], FP8, kind="ExternalInput")
    wo = nc.dram_tensor("wo", [GW, D], BF16, kind="ExternalInput")
    msk = nc.dram_tensor("msk", [P, P], BF16, kind="ExternalInput")
    out = nc.dram_tensor("out", [T, D], BF16, kind="ExternalOutput")

    with TileContext(nc) as tc:
        cms = []

        def pool(name, bufs, space=None):
            kw = {"space": space} if space else {}
            cm = tc.tile_pool(name=name, bufs=bufs, **kw)
            cms.append(cm)
            return cm.__enter__()

        big = pool("big", 1)
        ppool = pool("pp", 12)
        schp = pool("schp", 4)
        xkq = pool("xkq", 4)
        xvp = pool("xvp", 4)
        aop = pool("aop", 2)
        obp = pool("obp", 3)
        sp = pool("sp", 2, MemorySpace.PSUM)    # [128,1024] f32 -> 2 banks x2
        avp = pool("avp", 1, MemorySpace.PSUM)  # av 1 bank + dn 1 bank
        psp = pool("psp", 2, MemorySpace.PSUM)  # [128,512] f32 -> 1 bank x2

        kts = [big.tile([P, T], FP8, name=f"kt{j}") for j in range(4)]
        qts = [big.tile([P, 2, T], FP8, name=f"qt{j}") for j in range(4)]
        vsb = big.tile([P, NTB, HG * 65], BF16, name="vsb")
        wsb = {}
        for nm in ("wq", "wk", "wv"):
            for lv in ("h", "l"):
                wsb[nm + lv] = big.tile([P, KD, GW], FP8, name=f"{nm}{lv}_sb")
        wo_sb = big.tile([P, 4, D], BF16, name="wo_sb")
        mask_sb = big.tile([P, P], BF16, name="mask_sb")
        aotT = [big.tile([P, 4, 512], BF16, name=f"aotT{c}") for c in range(4)]

        vones = vsb.rearrange("p tb (h m) -> p tb h m", h=HG)[:, :, :, 64:65]
        nc.vector.memset(vones.bitcast(U16), 0x3F80)

        lo, hi = slice(0, 64), slice(64, 128)

        xk_t, xq_t, xv_t = {}, {}, {}

        def dma_x(nm, store, r, pool_, tag):
            th = pool_.tile([P, KD, 512], FP8, name=f"x{tag}h", tag=tag)
            tl = pool_.tile([P, KD, 512], FP8, name=f"x{tag}l", tag=tag)
            for t, lv in ((th, "h"), (tl, "l")):
                nc.sync.dma_start(
                    t,
                    xs_in[nm + lv].rearrange("(ko p) t -> p ko t", p=P)[
                        :, :, r * 512:(r + 1) * 512
                    ],
                )
            store[r] = (th, tl)

        def dma_w(nm, lv, eng=None):
            (eng or nc.sync).dma_start(
                wsb[nm + lv],
                ws_in[nm + lv].rearrange("(ko p) j -> p ko j", p=P),
            )

        dma_w("wk", "h")
        dma_w("wk", "l")
        dma_x("xk", xk_t, 0, xkq, "xk")
        dma_w("wq", "h")
        dma_w("wq", "l")
        dma_x("xq", xq_t, 0, xkq, "xq")
        nc.sync.dma_start(mask_sb, msk[:, :])
        dma_w("wv", "h")
        dma_w("wv", "l")
        dma_x("xv", xv_t, 0, xvp, "xv")

        # ---- filler steps: (need, pull, kind, fn) ----
        steps = []

        def kq_step(wh_sb, wl_sb, xst, dst, ch, jb, is_q):
            def fn():
                xht, xlt = xst[ch // 2]
                co = (ch % 2) * 256
                pst = psp.tile([P, GW], F32, name="ps_kq", tag="ps")
                ps = pst[:, 0:256]
                n = 0
                for kdp in range(4):
                    kk = slice(2 * kdp, 2 * kdp + 2)
                    for wsl, xsl in (
                        (wh_sb[:, kk, jb * P:(jb + 1) * P], xht[:, kk, co:co + 256]),
                        (wl_sb[:, kk, jb * P:(jb + 1) * P], xht[:, kk, co:co + 256]),
                        (wh_sb[:, kk, jb * P:(jb + 1) * P], xlt[:, kk, co:co + 256]),
                    ):
                        nc.tensor.matmul(
                            ps, wsl, xsl, start=(n == 0), stop=(n == 11),
                            perf_mode=DR, skip_group_check=True,
                        )
                        n += 1
                cs = slice(ch * 256, (ch + 1) * 256)
                if is_q:
                    nc.vector.tensor_copy(dst[jb][:, 0, cs], ps)
                    nc.vector.scalar_tensor_tensor(
                        out=dst[jb][:, 1, cs], in0=ps, scalar=1.0,
                        in1=dst[jb][:, 0, cs],
                        op0=mybir.AluOpType.mult, op1=mybir.AluOpType.subtract,
                    )
                else:
                    nc.vector.tensor_copy(dst[jb][:, cs], ps)
            return fn

        def v_step(tb):
            def fn():
                xht, xlt = xv_t[tb // 4]
                co = (tb % 4) * P
                ps = psp.tile([P, GW], F32, name="ps_v", tag="ps")
                n = 0
                for kdp in range(4):
                    kk = slice(2 * kdp, 2 * kdp + 2)
                    for xsl, wsl in (
                        (xht[:, kk, co:co + P], wsb["wvh"][:, kk, :]),
                        (xlt[:, kk, co:co + P], wsb["wvh"][:, kk, :]),
                        (xht[:, kk, co:co + P], wsb["wvl"][:, kk, :]),
                    ):
                        nc.tensor.matmul(
                            ps, xsl, wsl, start=(n == 0), stop=(n == 11),
                            perf_mode=DR, skip_group_check=True,
                        )
                        n += 1
                nc.vector.tensor_copy(
                    vsb[:, tb, :].rearrange("p (h m) -> p h m", h=HG)[:, :, 0:64],
                    ps.rearrange("p (h m) -> p h m", h=HG),
                )
            return fn

        def o_step(tb, oc):
            def fn():
                if oc == 0:
                    ob_t[tb % 2] = obp.tile([P, D], BF16, name="ob", tag="ob")
                ob = ob_t[tb % 2]
                c = tb // 4
                ps = psp.tile([P, GW], F32, name="ps_o", tag="ps")
                for jb in range(4):
                    nc.tensor.matmul(
                        ps, aotT[c][:, jb, (tb % 4) * P:(tb % 4 + 1) * P],
                        wo_sb[:, jb, oc * GW:(oc + 1) * GW],
                        start=(jb == 0), stop=(jb == 3),
                    )
                # 1/WS undoes the host-side V-weight prescale
                if tb >= 12:
                    nc.scalar.mul(ob[:, oc * GW:(oc + 1) * GW], ps, 1.0 / WS)
                else:
                    nc.vector.tensor_scalar(
                        out=ob[:, oc * GW:(oc + 1) * GW], in0=ps,
                        scalar1=1.0 / WS, scalar2=None, op0=mybir.AluOpType.mult,
                    )
                nc.sync.dma_start(
                    out[tb * P:(tb + 1) * P, oc * GW:(oc + 1) * GW],
                    ob[:, oc * GW:(oc + 1) * GW],
                )
            return fn

        ob_t = {}

        def wo_dma():
            nc.sync.dma_start(wo_sb, wo.rearrange("(jb p) o -> p jb o", p=P))

        v_idx, o_idx = {}, {}
        for u in range(16):
            r, pr = u // 4, u % 4
            if pr == 1 and r <= 2:
                steps.append((4 * r + 2, u - 1, "x",
                              (lambda r=r: dma_x("xk", xk_t, r + 1, xkq, "xk"))))
                steps.append((4 * r + 3, u - 1, "x",
                              (lambda r=r: dma_x("xq", xq_t, r + 1, xkq, "xq"))))
                steps.append((4 * r + 4, u - 1, "x",
                              (lambda r=r: dma_x("xv", xv_t, r + 1, xvp, "xv"))))
            if u == 3:
                steps.append((u, 0, "x", wo_dma))
            if u == 0:
                for ch in (0, 1):
                    for jb in (0, 1):
                        steps.append((0, 0, "p", kq_step(wsb["wkh"], wsb["wkl"], xk_t, kts, ch, jb, False)))
                for ch in (0, 1):
                    steps.append((0, 0, "p", kq_step(wsb["wkh"], wsb["wkl"], xk_t, kts, ch, 2, False)))
                for ch in (0, 1):
                    steps.append((0, 0, "p", kq_step(wsb["wqh"], wsb["wql"], xq_t, qts, ch, 0, True)))
                for ch in (0, 1):
                    steps.append((1, 0, "p", kq_step(wsb["wkh"], wsb["wkl"], xk_t, kts, ch, 3, False)))
            elif u < 4:
                for ch in (2 * r, 2 * r + 1):
                    steps.append((u, u, "p",
                                  kq_step(wsb["wqh"], wsb["wql"], xq_t, qts, ch, pr, True)))
            else:
                for ch in (2 * r, 2 * r + 1):
                    steps.append((u, u, "p",
                                  kq_step(wsb["wqh"], wsb["wql"], xq_t, qts, ch, pr, True)))
                for ch in (2 * r, 2 * r + 1):
                    steps.append((u, u, "p",
                                  kq_step(wsb["wkh"], wsb["wkl"], xk_t, kts, ch, pr, False)))
            if pr == 0:
                for tb in range(4 * r, 4 * r + 4):
                    v_idx[tb] = len(steps)
                    steps.append((17, u, "p", v_step(tb)))
            grp = []
            if r >= 1 and pr == 2:
                grp.append(4 * (r - 1))
            elif r >= 1 and pr == 3:
                grp.append(4 * (r - 1) + 1)
            elif r >= 2 and pr == 0:
                grp.append(4 * (r - 2) + 2)
            elif r >= 2 and pr == 1:
                grp.append(4 * (r - 2) + 3)
            if u == 15:
                grp += [10, 11]
            for tb in grp:
                for oc in (0, 1):
                    steps.append((u, u, "o", o_step(tb, oc)))
        for tb in range(12, 16):
            for oc in (0, 1):
                o_idx[(tb, oc)] = len(steps)
                steps.append((18, 18, "o", o_step(tb, oc)))

        emitted = [False] * len(steps)
        head = [0]

        def emit_step(i):
            if not emitted[i]:
                emitted[i] = True
                steps[i][3]()

        def ensure_v(tb):
            for t in range(tb + 1):
                emit_step(v_idx[t])

        def drain_force(maxneed):
            while head[0] < len(steps) and emitted[head[0]]:
                head[0] += 1
            i = head[0]
            while i < len(steps):
                if not emitted[i] and steps[i][0] <= maxneed:
                    emitted[i] = True
                    steps[i][3]()
                elif not emitted[i] and steps[i][0] > maxneed + 4:
                    break
                i += 1

        def drain_pull(u, limit):
            while head[0] < len(steps) and emitted[head[0]]:
                head[0] += 1
            n, i = 0, head[0]
            scanned = 0
            while i < len(steps) and n < limit and scanned < 80:
                if not emitted[i] and steps[i][1] <= u:
                    emitted[i] = True
                    steps[i][3]()
                    n += 1
                scanned += 1
                i += 1

        # ---- attention units, chunk-major; AV lag queue crosses units ----
        scale = float(DH) ** -0.5 / (WS * WS)
        import math
        SCH_SIGMA = 0.0430
        SCH_C = scale * math.log2(math.e) * (2 ** 23)
        SCH_D = (127.0 - SCH_SIGMA) * (2 ** 23)
        pend = []       # (unit, kb, F, d0, pp)
        ctx = {}        # unit -> state

        def emit_av(e):
            uu, kb, F, d0, pp = e
            cx = ctx[uu]
            ensure_v(kb)
            if cx["av"] is None:
                cx["av"] = avp.tile([P, 8, 64], F32, name="av", tag="av")
                cx["dn"] = avp.tile([P, 8], F32, name="dn", tag="dn")
            av, dn, c, pr = cx["av"], cx["dn"], cx["c"], cx["pr"]
            j = kb - 4 * c
            ppv = pp
            for qb in range(max(j, 0), 4):
                last = (kb == 4 * c + qb) if cx["diag_last"] else (kb == cx["last"][qb])
                for h in range(2):
                    lhs = ppv[:, h, qb * P - d0:(qb + 1) * P - d0]
                    vcol = (2 * pr + h) * 65
                    nc.tensor.matmul(
                        av[:, 2 * qb + h, :], lhs, vsb[:, kb, vcol:vcol + 64],
                        start=not cx["started"], stop=last, skip_group_check=True,
                    )
                    cx["started"] = True
                    nc.tensor.matmul(
                        dn[:, 2 * qb + h:2 * qb + h + 1], lhs,
                        vsb[:, kb, vcol + 64:vcol + 65],
                        start=not cx["dn_started"], stop=last, skip_group_check=True,
                    )
                    cx["dn_started"] = True
            cx["left"] -= 1
            if cx["diag_last"] and j >= 0:
                emit_tail_qb(cx, j)
            if cx["left"] == 0:
                if not cx["diag_last"]:
                    emit_tail(uu)
                ctx.pop(uu)

        def alloc_tail(cx):
            if cx["rcp"] is None:
                cx["rcp"] = aop.tile([P, 4, 2, 1], F32, name="rcp", tag="rcp")
                cx["ao"] = aop.tile([P, 4, 2, 64], BF16, name="ao", tag="ao")

        def emit_tail_qb(cx, qb):
            # u=15 path: query block qb is final once its diagonal AV lands
            alloc_tail(cx)
            av, dn, rcp, ao = cx["av"], cx["dn"], cx["rcp"], cx["ao"]
            c, pr = cx["c"], cx["pr"]
            nc.vector.reciprocal_approx_fast(
                rcp[:, qb, :, :],
                dn[:, 2 * qb:2 * qb + 2].rearrange("p (n o) -> p n o", o=1),
            )
            nc.vector.tensor_tensor(
                out=ao[:, qb, :, :],
                in0=av[:, 2 * qb:2 * qb + 2, :].rearrange("p (a b) f -> p a b f", b=2)[:, 0],
                in1=rcp[:, qb, :, :].broadcast_to([P, 2, 64]),
                op=mybir.AluOpType.mult,
            )
            nc.sync.dma_start_transpose(
                aotT[c][:, pr, qb * P:(qb + 1) * P],
                ao[:, qb, :, :].rearrange("p a b -> p (a b)"),
            )
            emit_step(o_idx[(12 + qb, 0)])
            emit_step(o_idx[(12 + qb, 1)])

        def emit_tail(uu):
            cx = ctx[uu]
            alloc_tail(cx)
            av, dn, rcp, ao = cx["av"], cx["dn"], cx["rcp"], cx["ao"]
            c, pr = cx["c"], cx["pr"]
            nc.vector.reciprocal_approx_fast(
                rcp.rearrange("p a b o -> p (a b) o"),
                dn.rearrange("p (n o) -> p n o", o=1),
            )
            nc.vector.tensor_tensor(
                out=ao,
                in0=av.rearrange("p (a b) f -> p a b f", b=2),
                in1=rcp.broadcast_to([P, 4, 2, 64]),
                op=mybir.AluOpType.mult,
            )
            for qb in range(4):
                nc.sync.dma_start_transpose(
                    aotT[c][:, pr, qb * P:(qb + 1) * P],
                    ao[:, qb, :, :].rearrange("p a b -> p (a b)"),
                )

        for u in range(16):
            c, pr = u // 4, u % 4
            kt, qt = kts[pr], qts[pr]
            nblk = 4 * (c + 1)
            drain_force(u)

            if u < 15:
                kb_order = list(range(4 * c, nblk)) + list(range(0, 4 * c))
            else:
                kb_order = list(range(nblk))
            last = {}
            for kb in kb_order:
                j = kb - 4 * c
                for qb in range(max(j, 0), 4):
                    last[qb] = kb
            ctx[u] = {"av": None, "dn": None, "rcp": None, "ao": None,
                      "pr": pr, "c": c, "u": u, "left": nblk, "last": last,
                      "started": False, "dn_started": False,
                      "diag_last": u == 15}

            for ki, kb in enumerate(kb_order):
                j = kb - 4 * c
                F = 512 if j < 0 else 512 - 128 * j
                d0 = 0 if j < 0 else 128 * j
                s_pair = sp.tile([P, 1024], F32, name="s_pair", tag="sp")
                ks = slice(kb * P, (kb + 1) * P)
                qs = slice(512 * c + d0, 512 * c + d0 + F)
                kt_lo = kt[lo, ks].rearrange("p (o f) -> p o f", o=1).broadcast_to([64, 2, P])
                kt_hi = kt[hi, ks].rearrange("p (o f) -> p o f", o=1).broadcast_to([64, 2, P])
                nc.tensor.matmul(
                    s_pair[:, 0:F], kt_lo, qt[lo, :, qs], start=True, stop=True,
                    perf_mode=DR, skip_group_check=True,
                )
                nc.tensor.matmul(
                    s_pair[:, 512:512 + F], kt_hi, qt[hi, :, qs],
                    start=True, stop=True, perf_mode=DR, skip_group_check=True,
                )
                pp = ppool.tile([P, 1024], BF16, name="p_pair", tag="pp")
                sv = s_pair.rearrange("p (h q) -> p h q", h=2)[:, :, 0:F]
                pv = pp.rearrange("p (h q) -> p h q", h=2)[:, :, 0:F]
                schv = None
                if j < 0 and u >= 8 and ki >= 4 and (ki - 4) % 3 == 0:
                    # Schraudolph fast-exp off Act: y = s*C + D on DVE,
                    # bits<<8 on Pool; probs consumed as the strided bf16
                    # (high-half) view of the f32 words - no convert pass.
                    y32 = schp.tile([P, 1024], F32, name="y32", tag="y32")
                    yi = schp.tile([P, 1024], mybir.dt.int32, name="yi", tag="yi")
                    nc.vector.tensor_scalar(
                        out=y32.rearrange("p (h q) -> p h q", h=2),
                        in0=sv, scalar1=SCH_C, scalar2=SCH_D,
                        op0=mybir.AluOpType.mult, op1=mybir.AluOpType.add,
                    )
                    nc.gpsimd.tensor_copy(yi, y32)
                    schv = yi.bitcast(BF16).rearrange(
                        "p (h q two) -> p h q two", h=2, two=2)[:, :, :, 1]
                if schv is None:
                    nc.scalar.activation(
                        pv, sv, mybir.ActivationFunctionType.Exp, scale=scale,
                    )
                if j >= 0:
                    # triangular mask on the 128x128 diagonal block
                    ppm = pp.rearrange("p (h q) -> p h q", h=2)[:, :, 0:128]
                    nc.gpsimd.tensor_tensor(
                        out=ppm, in0=ppm,
                        in1=mask_sb.rearrange("p (o f) -> p o f", o=1).broadcast_to([P, 2, P]),
                        op=mybir.AluOpType.mult,
                    )
                drain_pull(u, PULL)
                pend.append((u, kb, F, d0, schv if schv is not None else
                             pp.rearrange("p (h q) -> p h q", h=2)))
                lag_u = LAG if u < 15 else 3
                if len(pend) > lag_u:
                    emit_av(pend.pop(0))
                if u == 15 and kb >= 12:
                    for _ in range(2):
                        if pend:
                            emit_av(pend.pop(0))

        while pend:
            emit_av(pend.pop(0))
            drain_pull(16, 1)
        drain_force(99)

        for cm in reversed(cms):
            cm.__exit__(None, None, None)

    nc.finalize()
    return nc


def _split8(a):
    hi = np.asarray(a, np.float32).astype(ml_dtypes.float8_e4m3)
    lo = (np.asarray(a, np.float32) - hi.astype(np.float32)).astype(
        ml_dtypes.float8_e4m3)
    return hi, lo


def make_in_maps(q, k, v, Wq, Wk, Wv, Wo):
    mask_bf = (
        np.arange(P)[None, :] >= np.arange(P)[:, None]
    ).astype(ml_dtypes.bfloat16)
    xs = []
    for b in range(B):
        d = {}
        for nm, t in (("xq", q), ("xk", k), ("xv", v)):
            h, l = _split8(np.ascontiguousarray(t[b].T))
            d[nm + "h"], d[nm + "l"] = h, l
        xs.append(d)
    ws = []
    for g in range(2):
        hs = slice(g * GW, (g + 1) * GW)
        d = {}
        for nm, W in (("wq", Wq), ("wk", Wk), ("wv", Wv)):
            h, l = _split8(np.ascontiguousarray(W[hs, :].T) * WS)
            d[nm + "h"], d[nm + "l"] = h, l
        d["wo"] = np.ascontiguousarray(Wo[:, hs].T).astype(ml_dtypes.bfloat16)
        ws.append(d)
    return [
        {**xs[c // 2], **ws[c % 2], "msk": mask_bf} for c in range(N_CORES)
    ]


_NC_CACHE = None


def kernel(q, k, v, mask, Wq, Wk, Wv, Wo):
    global _NC_CACHE
    if _NC_CACHE is None:
        _NC_CACHE = build_nc()
    nc = _NC_CACHE

    from concourse.bass_utils import run_bass_kernel_spmd

    q, k, v = np.asarray(q), np.asarray(k), np.asarray(v)
    Wq, Wk, Wv, Wo = (np.asarray(t) for t in (Wq, Wk, Wv, Wo))
    in_maps = make_in_maps(q, k, v, Wq, Wk, Wv, Wo)

    r = run_bass_kernel_spmd(nc, in_maps, core_ids=list(range(N_CORES)))
    parts = [np.asarray(r.results[c]["out"], dtype=np.float32) for c in range(N_CORES)]
    y = np.stack([parts[2 * b] + parts[2 * b + 1] for b in range(B)])
    return y
